# revision 9
# baseline (speedup 1.0000x reference)
"""GCN layer (SpMM + Linear + LayerNorm + ReLU) on 8 Trainium2 NeuronCores.

Strategy (node sharding, streaming matmul-gather + CCE scatter-add):
  - Core c owns dest rows [c*rpc, (c+1)*rpc). Its edges are assigned to 100
    scatter calls of 2048 lanes such that dest rows are UNIQUE within each
    call (the CCE scatter-add loses colliding read-modify-writes inside one
    call; max dest degree ~36 << 100 calls). Runs of equal source col stay
    contiguous, so each 64-lane group still has <= 48 distinct source rows,
    which form the group's private x-window (host-assembled, streamed bf16).
  - Gather: per group, ONE TensorE matmul  Xg[64 lanes, 64f] = G3.T @ xwin
    with G3 [48, 64] a host-built fp8 one-hot (col-rank -> lane). No per-edge
    DMA descriptors on the gather side.
  - Messages: PSUM -> SBUF bf16 copy (scalar engine), then one DVE multiply
    by per-edge val (broadcast over the 64 feature columns).
  - Aggregation: gpsimd dma_scatter_add (SBUF parity-split CCE add) into
    SBUF-resident agg tables keyed by local dest row: partition=d&127,
    parity table=bit7(d), free col=d>>8. 128B descriptors (64 bf16);
    dynamic_dma_scratch_size=98304 so 2048-desc calls fit the SWDGE ring.
  - Epilogue per 128-row tile: PE transpose (bf16) -> [65,128] lhsT with ones
    row; Linear via wtb matmul (centering folded into weights); var from
    Square-activation accumulate; out = relu(h * rstd) on the gamma=1/beta=0
    fast path (general path uses vector ops).
"""

import numpy as np
import ml_dtypes

N_NODES = 100000
DIM = 64
LN_EPS = 1e-5
NCORES = 8

GL = 64          # lanes (edges) per group
WIN = 48         # x-window rows per group (max distinct cols per group)
NCALLS = 100     # scatter calls (dest rows unique within each call)
CAP = 2048       # lanes per scatter call
GPCALL = CAP // GL           # 32 groups per call
CHC = 4          # calls per DMA chunk


def _host_prep(edge_row, edge_col, edge_val, n_nodes, ncores):
    rpc = n_nodes // ncores
    G = NCALLS * GPCALL
    nlanes = NCALLS * CAP

    er = np.asarray(edge_row).astype(np.int64)
    ec = np.asarray(edge_col).astype(np.int64)
    ev = np.asarray(edge_val).astype(np.float32)
    core = er // rpc

    percore = []
    for c in range(ncores):
        m = core == c
        cols = ec[m]
        dests = (er[m] - c * rpc).astype(np.int64)
        vals = ev[m].astype(np.float64)
        # merge duplicate (col, dest) pairs (sum vals)
        order = np.lexsort((dests, cols))
        cols, dests, vals = cols[order], dests[order], vals[order]
        key_new = np.concatenate(
            [[True], (np.diff(cols) != 0) | (np.diff(dests) != 0)]
        )
        gid = np.cumsum(key_new) - 1
        vsum = np.zeros(gid[-1] + 1, np.float64)
        np.add.at(vsum, gid, vals)
        cols = cols[key_new]
        dests = dests[key_new]
        vals = vsum.astype(np.float32)
        E = len(cols)
        assert E <= nlanes, (E, nlanes)

        # runs of equal col
        starts = np.nonzero(np.concatenate([[True], np.diff(cols) != 0]))[0]
        ends = np.concatenate([starts[1:], [E]])

        # greedy: place each run into a call with no dest collision
        lane_of = np.empty(E, np.int64)      # global lane id (call*CAP + pos)
        rank_of = np.empty(E, np.int64)      # window rank within group
        call_lanes = [0] * NCALLS
        call_dest = [set() for _ in range(NCALLS)]
        grp_l = [0] * NCALLS
        grp_d = [0] * NCALLS
        wcol = np.zeros(G * WIN, np.int64)   # window col per (group, rank)
        ptr = 0

        def place_edges(ci, idxs):
            # append edges idxs (same col) to call ci, updating group state
            nonlocal wcol
            l, d = grp_l[ci], grp_d[ci]
            pos = call_lanes[ci]
            col = cols[idxs[0]]
            i = 0
            while i < len(idxs):
                if l == 0:
                    d = 0
                if d + 1 > WIN:
                    # pad group to boundary with dummy lanes
                    pos += GL - l
                    l, d = 0, 0
                g = ci * GPCALL + pos // GL
                d += 1
                wcol[g * WIN + d - 1] = col
                take = min(len(idxs) - i, GL - l)
                for k in range(take):
                    e = idxs[i + k]
                    lane_of[e] = ci * CAP + pos
                    rank_of[e] = d - 1
                    pos += 1
                l += take
                i += take
                if l == GL:
                    l, d = 0, 0
            call_lanes[ci] = pos
            grp_l[ci], grp_d[ci] = l, d

        def fits(ci, n, dset):
            if call_lanes[ci] + n > CAP:
                return False
            if call_dest[ci] & dset:
                return False
            # group/window feasibility (account for possible padding)
            l, d = grp_l[ci], grp_d[ci]
            rem = n
            extra = 0
            while rem > 0:
                if l == 0:
                    d = 0
                if d + 1 > WIN:
                    extra += GL - l
                    l, d = 0, 0
                d += 1
                take = min(rem, GL - l)
                l += take
                rem -= take
                if l == GL:
                    l, d = 0, 0
            return call_lanes[ci] + n + extra <= CAP

        for s, e in zip(starts, ends):
            idxs = np.arange(s, e)
            dset = set(dests[s:e].tolist())
            placed = False
            for t in range(NCALLS):
                ci = (ptr + t) % NCALLS
                if fits(ci, e - s, dset):
                    call_dest[ci] |= dset
                    place_edges(ci, idxs)
                    placed = True
                    ptr = (ptr + 1) % NCALLS
                    break
            if not placed:
                for e1 in idxs:
                    d1 = int(dests[e1])
                    pl = False
                    for t in range(NCALLS):
                        ci = (ptr + t) % NCALLS
                        if fits(ci, 1, {d1}):
                            call_dest[ci].add(d1)
                            place_edges(ci, np.asarray([e1]))
                            pl = True
                            ptr = (ptr + 1) % NCALLS
                            break
                    assert pl, "scatter call overflow"

        percore.append({
            "E": E, "cols": cols, "dests": dests, "vals": vals,
            "lane_of": lane_of, "rank_of": rank_of, "wcol": wcol,
        })

    ntiles = (rpc + 255) // 256
    return {
        "rpc": rpc, "G": G, "nlanes": nlanes,
        "ntiles": ntiles, "nrows_pad": ntiles * 256,
        "percore": percore,
    }


def _host_arrays(sched, x):
    """Per-core input arrays for the SPMD program."""
    rpc = sched["rpc"]
    G = sched["G"]
    nlanes = sched["nlanes"]
    npairs = G // 2
    dummy0 = rpc + 8

    xbf = x.astype(ml_dtypes.bfloat16)
    out = []
    for pc in sched["percore"]:
        # xin [128, npairs*64] bf16: pair k cols [64k,64k+64); parts[0:WIN]
        # group 2k window rows (x rows), parts[64:64+WIN] group 2k+1
        xin = np.zeros((128, npairs * 64), ml_dtypes.bfloat16)
        wrows = xbf[pc["wcol"]].reshape(G, WIN, 64)
        v = xin[0:WIN].reshape(WIN, npairs, 64)
        v[:] = wrows[0::2].transpose(1, 0, 2)
        v2 = xin[64:64 + WIN].reshape(WIN, npairs, 64)
        v2[:] = wrows[1::2].transpose(1, 0, 2)

        # g3 [128, npairs*64] fp8 one-hot
        g3 = np.zeros((128, npairs * 64), ml_dtypes.float8_e4m3)
        li = pc["lane_of"]
        gg = li // GL
        lane = li % GL
        part = (gg % 2) * 64 + pc["rank_of"]
        colix = (gg // 2) * 64 + lane
        g3[part, colix] = 1.0

        # lane-indexed dest + val (pad lanes -> dummy rows, val 0)
        dest = np.full(nlanes, 0, np.int64)
        dest[:] = dummy0 + (np.arange(nlanes) % 32)
        val = np.zeros(nlanes, np.float32)
        dest[li] = pc["dests"]
        val[li] = pc["vals"]

        # idx wrap [128, nlanes/16] int16 (16-partition wrap, replicated x8)
        idxw = np.zeros((128, nlanes // 16), np.int16)
        w = dest.astype(np.int16).reshape(nlanes // 16, 16).T
        idxw[:] = np.tile(w, (8, 1))

        # val [128, NCALLS*16] bf16: [p, call*16+s] = val[call*CAP+128*s+p]
        vv = val.reshape(NCALLS, 16, 128).transpose(2, 0, 1).reshape(128, NCALLS * 16)
        valw = vv.astype(ml_dtypes.bfloat16)

        out.append({"xin": xin, "g3": g3, "idx": idxw, "val": valw})
    return out


def _make_bacc(ncores):
    from concourse import bacc
    return bacc.Bacc(
        "TRN2", target_bir_lowering=False, debug=False, num_devices=ncores,
        dynamic_dma_scratch_size=98304,
    )


def _build_program(nc, sched, n_nodes, fastpath, debug_agg=False):
    from contextlib import ExitStack
    import concourse.bass as bass
    import concourse.tile as tile
    from concourse import mybir

    f32 = mybir.dt.float32
    bf16 = mybir.dt.bfloat16
    fp8 = mybir.dt.float8e4
    i16 = mybir.dt.int16
    AF = mybir.ActivationFunctionType

    rpc = sched["rpc"]
    G = sched["G"]
    nlanes = sched["nlanes"]
    npairs = G // 2
    nrows_pad = sched["nrows_pad"]

    xind = nc.dram_tensor("xin", [128, npairs * 64], bf16, kind="ExternalInput")
    g3d = nc.dram_tensor("g3", [128, npairs * 64], fp8, kind="ExternalInput")
    idxd = nc.dram_tensor("idx", [128, nlanes // 16], i16, kind="ExternalInput")
    vald = nc.dram_tensor("val", [128, NCALLS * 16], bf16, kind="ExternalInput")
    wtbd = nc.dram_tensor("wtb", [DIM + 1, DIM], f32, kind="ExternalInput")
    gbd = nc.dram_tensor("gb", [2, DIM], f32, kind="ExternalInput")
    identd = nc.dram_tensor("ident", [128, 128], bf16, kind="ExternalInput")
    outd = nc.dram_tensor("out", [nrows_pad, DIM], f32, kind="ExternalOutput")

    nchunks = NCALLS // CHC
    PAIR_C = CHC * GPCALL // 2          # pairs per chunk (64)

    with tile.TileContext(nc) as tc, ExitStack() as ctx:
        singles = ctx.enter_context(tc.tile_pool(name="singles", bufs=1))
        xpool = ctx.enter_context(tc.tile_pool(name="xp", bufs=3))
        gpool = ctx.enter_context(tc.tile_pool(name="gp", bufs=3))
        ipool = ctx.enter_context(tc.tile_pool(name="ip", bufs=3))
        vpool = ctx.enter_context(tc.tile_pool(name="vp", bufs=3))
        mpool = ctx.enter_context(tc.tile_pool(name="mp", bufs=3))
        pagg = ctx.enter_context(tc.tile_pool(name="pagg", bufs=4, space="PSUM"))
        ptr = ctx.enter_context(tc.tile_pool(name="ptr", bufs=2, space="PSUM"))
        ph = ctx.enter_context(tc.tile_pool(name="ph", bufs=2, space="PSUM"))
        lpool = ctx.enter_context(tc.tile_pool(name="lp", bufs=2))

        wtb_s = singles.tile([DIM + 1, DIM], f32)
        nc.sync.dma_start(out=wtb_s[:], in_=wtbd[:])
        ident_s = singles.tile([128, 128], bf16)
        nc.sync.dma_start(out=ident_s[:], in_=identd[:])
        eps_s = singles.tile([128, 1], f32)
        nc.vector.memset(eps_s[:], LN_EPS)
        if not fastpath:
            gam_s = singles.tile([128, DIM], f32)
            bet_s = singles.tile([128, DIM], f32)
            gsrc = gbd.ap()
            nc.sync.dma_start(
                out=gam_s[:],
                in_=bass.AP(tensor=gsrc.tensor, offset=0, ap=[[0, 128], [1, DIM]]),
            )
            nc.sync.dma_start(
                out=bet_s[:],
                in_=bass.AP(tensor=gsrc.tensor, offset=DIM, ap=[[0, 128], [1, DIM]]),
            )

        # SBUF-resident agg tables (parity split), zeroed
        nblk = nrows_pad // 256
        agg_e = singles.tile([128, nblk, DIM], bf16)
        agg_o = singles.tile([128, nblk, DIM], bf16)
        nc.vector.memset(agg_e[:], 0.0)
        nc.vector.memset(agg_o[:], 0.0)

        for ch in range(nchunks):
            c0 = ch * PAIR_C * 64
            c1 = (ch + 1) * PAIR_C * 64
            xt = xpool.tile([128, PAIR_C * 64], bf16, tag="xt")
            nc.sync.dma_start(out=xt[0:WIN, :], in_=xind[0:WIN, c0:c1])
            nc.sync.dma_start(out=xt[64:64 + WIN, :], in_=xind[64:64 + WIN, c0:c1])
            gt = gpool.tile([128, PAIR_C * 64], fp8, tag="gt")
            nc.sync.dma_start(out=gt[0:WIN, :], in_=g3d[0:WIN, c0:c1])
            nc.sync.dma_start(out=gt[64:64 + WIN, :], in_=g3d[64:64 + WIN, c0:c1])
            it = ipool.tile([128, CHC * CAP // 16], i16, tag="it")
            nc.sync.dma_start(
                out=it[:],
                in_=idxd[:, ch * CHC * CAP // 16:(ch + 1) * CHC * CAP // 16],
            )
            vt = vpool.tile([128, CHC * 16], bf16, tag="vt")
            nc.sync.dma_start(
                out=vt[:], in_=vald[:, ch * CHC * 16:(ch + 1) * CHC * 16]
            )

            for q in range(CHC):
                msgs = mpool.tile([128, 16, DIM], bf16, tag="msgs")
                for b in range(2):          # psum banks per call
                    ps = pagg.tile([128, 8, DIM], f32, tag="ps")
                    for k in range(8):      # pair within bank
                        pr = q * 16 + b * 8 + k   # pair within chunk
                        for h in range(2):  # group parity
                            po = 64 * h
                            nc.tensor.matmul(
                                out=ps[po:po + 64, k, :],
                                lhsT=gt[po:po + WIN, pr * 64:(pr + 1) * 64],
                                rhs=xt[po:po + WIN, pr * 64:(pr + 1) * 64],
                                start=True,
                                stop=True,
                                skip_group_check=True,
                                tile_position=(po, po),
                            )
                    nc.scalar.copy(
                        out=msgs[:, b * 8:(b + 1) * 8, :], in_=ps[:]
                    )

                _vs = vt[:, q * 16:(q + 1) * 16]
                val_b = bass.AP(
                    tensor=_vs.tensor,
                    offset=_vs.offset,
                    ap=list(_vs.ap) + [[0, DIM]],
                )
                nc.vector.tensor_mul(out=msgs[:], in0=msgs[:], in1=val_b)

                nc.gpsimd.dma_scatter_add(
                    out_ap=agg_e[:],
                    out_ap_other=agg_o[:],
                    in_ap=msgs[:],
                    idxs_ap=it[:, q * (CAP // 16):(q + 1) * (CAP // 16)],
                    num_idxs=CAP,
                    num_idxs_reg=CAP,
                    elem_size=DIM,
                    sbuf_tokens_per_rank=128,
                    parity_reg=0,
                )

        if debug_agg:
            agge_d = nc.dram_tensor(
                "dbg_agge", [128, nblk * DIM], bf16, kind="ExternalOutput"
            )
            aggo_d = nc.dram_tensor(
                "dbg_aggo", [128, nblk * DIM], bf16, kind="ExternalOutput"
            )
            nc.sync.dma_start(
                out=agge_d[:], in_=agg_e[:].rearrange("p b f -> p (b f)")
            )
            nc.sync.dma_start(
                out=aggo_d[:], in_=agg_o[:].rearrange("p b f -> p (b f)")
            )

        # ---- epilogue: Linear + LayerNorm + ReLU per 128-row tile ----
        ntile128 = nrows_pad // 128
        ssq = singles.tile([128, ntile128], f32)
        rstd = singles.tile([128, ntile128], f32)
        sq_scratch = singles.tile([128, DIM], f32)

        BT = 7  # tiles per batch (98 tiles = 14 * 7)
        nbatch = ntile128 // BT
        assert nbatch * BT == ntile128, (ntile128, BT)
        for bt in range(nbatch):
            hps = ph.tile([128, BT, DIM], f32, tag="h")
            o_t = lpool.tile([128, BT, DIM], f32, tag="ot")
            for k in range(BT):
                t = bt * BT + k
                g, par = t // 2, t % 2
                src = agg_e if par == 0 else agg_o
                tp = ptr.tile([64, 128], bf16, tag="tp")
                nc.tensor.transpose(tp[:], src[:, g, :], ident_s[:])
                abT = lpool.tile([DIM + 1, 128], f32, tag="abT")
                nc.scalar.copy(out=abT[0:DIM, :], in_=tp[:])
                nc.vector.memset(abT[DIM:DIM + 1, :], 1.0)
                nc.tensor.matmul(
                    out=hps[:, k, :],
                    lhsT=abT[:],
                    rhs=wtb_s[:],
                    start=True,
                    stop=True,
                    skip_group_check=True,
                )
                nc.scalar.activation(
                    out=sq_scratch[:],
                    in_=hps[:, k, :],
                    func=AF.Square,
                    accum_out=ssq[:, t:t + 1],
                )
            nc.scalar.activation(
                out=rstd[:, bt * BT:(bt + 1) * BT],
                in_=ssq[:, bt * BT:(bt + 1) * BT],
                func=AF.Sqrt,
                bias=eps_s[:, :],
                scale=1.0 / DIM,
            )
            nc.vector.reciprocal(
                out=rstd[:, bt * BT:(bt + 1) * BT],
                in_=rstd[:, bt * BT:(bt + 1) * BT],
            )
            for k in range(BT):
                t = bt * BT + k
                if fastpath:
                    nc.scalar.activation(
                        out=o_t[:, k, :],
                        in_=hps[:, k, :],
                        func=AF.Relu,
                        scale=rstd[:, t:t + 1],
                    )
                else:
                    nc.scalar.mul(
                        out=o_t[:, k, :], in_=hps[:, k, :], mul=rstd[:, t:t + 1]
                    )
                    nc.vector.tensor_mul(
                        out=o_t[:, k, :], in0=o_t[:, k, :], in1=gam_s[:, :]
                    )
                    nc.vector.tensor_add(
                        out=o_t[:, k, :], in0=o_t[:, k, :], in1=bet_s[:, :]
                    )
                    nc.vector.tensor_scalar_max(
                        out=o_t[:, k, :], in0=o_t[:, k, :], scalar1=0.0
                    )
            dst = outd[bt * BT * 128:(bt + 1) * BT * 128, :].rearrange(
                "(k p) f -> p k f", p=128
            )
            nc.sync.dma_start(out=dst, in_=o_t[:])


def _execute(inputs, n_nodes=N_NODES, ncores=NCORES, trace=False, trace_cores=None):
    from concourse.bass_utils import run_bass_kernel_spmd

    x = np.asarray(inputs["x"], np.float32)
    W = np.asarray(inputs["W"], np.float32)
    b = np.asarray(inputs["b"], np.float32)
    gamma = np.asarray(inputs["gamma"], np.float32)
    beta = np.asarray(inputs["beta"], np.float32)

    sched = _host_prep(
        inputs["edge_row"], inputs["edge_col"], inputs["edge_val"], n_nodes, ncores
    )
    arrs = _host_arrays(sched, x)

    WT = W.T.astype(np.float32)
    WTc = WT - WT.mean(axis=1, keepdims=True)
    bc = (b - b.mean()).astype(np.float32)
    wtb = np.concatenate([WTc, bc[None, :]], axis=0).astype(np.float32)
    gb = np.stack([gamma, beta], axis=0).astype(np.float32)
    ident = np.eye(128, dtype=ml_dtypes.bfloat16)

    fastpath = bool(np.all(gamma == 1.0) and np.all(beta == 0.0))

    nc = _make_bacc(ncores)
    _build_program(nc, sched, n_nodes, fastpath)
    nc.compile()

    in_maps = [
        {
            "xin": np.ascontiguousarray(arrs[c]["xin"]),
            "g3": np.ascontiguousarray(arrs[c]["g3"]),
            "idx": np.ascontiguousarray(arrs[c]["idx"]),
            "val": np.ascontiguousarray(arrs[c]["val"]),
            "wtb": wtb,
            "gb": gb,
            "ident": ident,
        }
        for c in range(ncores)
    ]
    r = run_bass_kernel_spmd(
        nc,
        in_maps,
        list(range(ncores)),
        trace=trace,
        trace_cores=trace_cores,
    )
    rpc = n_nodes // ncores
    out = np.concatenate(
        [r.results[c]["out"][:rpc] for c in range(ncores)], axis=0
    )
    return out.astype(np.float32), r


def kernel(**inputs):
    out, _ = _execute(inputs)
    return out


# revision 18
# speedup vs baseline: 1.6171x; 1.6171x over previous
"""GCN layer (SpMM + Linear + LayerNorm + ReLU) on 8 Trainium2 NeuronCores.

Strategy (node sharding, streaming matmul-gather + CCE scatter-add):
  - Core c owns dest rows [c*rpc, (c+1)*rpc). Its edges are assigned to 100
    scatter calls of 2048 lanes such that dest rows are UNIQUE within each
    call (the CCE scatter-add loses colliding read-modify-writes inside one
    call; max dest degree ~36 << 100 calls). Runs of equal source col stay
    contiguous, so each 64-lane group still has <= 48 distinct source rows,
    which form the group's private x-window (host-assembled, streamed bf16).
  - Gather: per group, ONE TensorE matmul  Xg[64 lanes, 64f] = G3.T @ xwin
    with G3 [48, 64] a host-built fp8 one-hot (col-rank -> lane). No per-edge
    DMA descriptors on the gather side.
  - Messages: PSUM -> SBUF bf16 copy (scalar engine), then one DVE multiply
    by per-edge val (broadcast over the 64 feature columns).
  - Aggregation: gpsimd dma_scatter_add (SBUF parity-split CCE add) into
    SBUF-resident agg tables keyed by local dest row: partition=d&127,
    parity table=bit7(d), free col=d>>8. 128B descriptors (64 bf16);
    dynamic_dma_scratch_size=98304 so 2048-desc calls fit the SWDGE ring.
  - Epilogue per 128-row tile: PE transpose (bf16) -> [65,128] lhsT with ones
    row; Linear via wtb matmul (centering folded into weights); var from
    Square-activation accumulate; out = relu(h * rstd) on the gamma=1/beta=0
    fast path (general path uses vector ops).
"""

import numpy as np
import ml_dtypes

N_NODES = 100000
DIM = 64
LN_EPS = 1e-5
NCORES = 8

GL = 64          # lanes (edges) per group
WIN = 48         # x-window rows per group (max distinct cols per group)
NCALLS = 100     # scatter calls (dest rows unique within each call)
CAP = 2048       # lanes per scatter call
GPCALL = CAP // GL           # 32 groups per call
CHC = 4          # calls per DMA chunk


def _host_prep(edge_row, edge_col, edge_val, n_nodes, ncores):
    rpc = n_nodes // ncores
    G = NCALLS * GPCALL
    nlanes = NCALLS * CAP

    er = np.asarray(edge_row).astype(np.int64)
    ec = np.asarray(edge_col).astype(np.int64)
    ev = np.asarray(edge_val).astype(np.float32)
    core = er // rpc

    percore = []
    for c in range(ncores):
        m = core == c
        cols = ec[m]
        dests = (er[m] - c * rpc).astype(np.int64)
        vals = ev[m].astype(np.float64)
        # merge duplicate (col, dest) pairs (sum vals)
        order = np.lexsort((dests, cols))
        cols, dests, vals = cols[order], dests[order], vals[order]
        key_new = np.concatenate(
            [[True], (np.diff(cols) != 0) | (np.diff(dests) != 0)]
        )
        gid = np.cumsum(key_new) - 1
        vsum = np.zeros(gid[-1] + 1, np.float64)
        np.add.at(vsum, gid, vals)
        cols = cols[key_new]
        dests = dests[key_new]
        vals = vsum.astype(np.float32)
        E = len(cols)
        assert E <= nlanes, (E, nlanes)

        # runs of equal col
        starts = np.nonzero(np.concatenate([[True], np.diff(cols) != 0]))[0]
        ends = np.concatenate([starts[1:], [E]])

        # greedy: place each run into a call with no dest collision
        lane_of = np.empty(E, np.int64)      # global lane id (call*CAP + pos)
        rank_of = np.empty(E, np.int64)      # window rank within group
        call_lanes = [0] * NCALLS
        call_dest = [set() for _ in range(NCALLS)]
        grp_l = [0] * NCALLS
        grp_d = [0] * NCALLS
        wcol = np.zeros(G * WIN, np.int64)   # window col per (group, rank)
        ptr = 0

        def place_edges(ci, idxs):
            # append edges idxs (same col) to call ci, updating group state
            nonlocal wcol
            l, d = grp_l[ci], grp_d[ci]
            pos = call_lanes[ci]
            col = cols[idxs[0]]
            i = 0
            while i < len(idxs):
                if l == 0:
                    d = 0
                if d + 1 > WIN:
                    # pad group to boundary with dummy lanes
                    pos += GL - l
                    l, d = 0, 0
                g = ci * GPCALL + pos // GL
                d += 1
                wcol[g * WIN + d - 1] = col
                take = min(len(idxs) - i, GL - l)
                for k in range(take):
                    e = idxs[i + k]
                    lane_of[e] = ci * CAP + pos
                    rank_of[e] = d - 1
                    pos += 1
                l += take
                i += take
                if l == GL:
                    l, d = 0, 0
            call_lanes[ci] = pos
            grp_l[ci], grp_d[ci] = l, d

        def fits(ci, n, dset):
            if call_lanes[ci] + n > CAP:
                return False
            if call_dest[ci] & dset:
                return False
            # group/window feasibility (account for possible padding)
            l, d = grp_l[ci], grp_d[ci]
            rem = n
            extra = 0
            while rem > 0:
                if l == 0:
                    d = 0
                if d + 1 > WIN:
                    extra += GL - l
                    l, d = 0, 0
                d += 1
                take = min(rem, GL - l)
                l += take
                rem -= take
                if l == GL:
                    l, d = 0, 0
            return call_lanes[ci] + n + extra <= CAP

        for s, e in zip(starts, ends):
            idxs = np.arange(s, e)
            dset = set(dests[s:e].tolist())
            placed = False
            for t in range(NCALLS):
                ci = (ptr + t) % NCALLS
                if fits(ci, e - s, dset):
                    call_dest[ci] |= dset
                    place_edges(ci, idxs)
                    placed = True
                    ptr = (ptr + 1) % NCALLS
                    break
            if not placed:
                for e1 in idxs:
                    d1 = int(dests[e1])
                    pl = False
                    for t in range(NCALLS):
                        ci = (ptr + t) % NCALLS
                        if fits(ci, 1, {d1}):
                            call_dest[ci].add(d1)
                            place_edges(ci, np.asarray([e1]))
                            pl = True
                            ptr = (ptr + 1) % NCALLS
                            break
                    assert pl, "scatter call overflow"

        percore.append({
            "E": E, "cols": cols, "dests": dests, "vals": vals,
            "lane_of": lane_of, "rank_of": rank_of, "wcol": wcol,
        })

    ntiles = (rpc + 255) // 256
    return {
        "rpc": rpc, "G": G, "nlanes": nlanes,
        "ntiles": ntiles, "nrows_pad": ntiles * 256,
        "percore": percore,
    }


def _host_arrays(sched, x):
    """Per-core input arrays for the SPMD program."""
    rpc = sched["rpc"]
    G = sched["G"]
    nlanes = sched["nlanes"]
    npairs = G // 2
    dummy0 = rpc + 8

    xbf = x.astype(ml_dtypes.bfloat16)
    out = []
    for pc in sched["percore"]:
        # xin [128, npairs*64] bf16: pair k cols [64k,64k+64); parts[0:WIN]
        # group 2k window rows (x rows), parts[64:64+WIN] group 2k+1
        xin = np.zeros((128, npairs * 64), ml_dtypes.bfloat16)
        wrows = xbf[pc["wcol"]].reshape(G, WIN, 64)
        v = xin[0:WIN].reshape(WIN, npairs, 64)
        v[:] = wrows[0::2].transpose(1, 0, 2)
        v2 = xin[64:64 + WIN].reshape(WIN, npairs, 64)
        v2[:] = wrows[1::2].transpose(1, 0, 2)

        # g3 [128, npairs*64] fp8 one-hot
        g3 = np.zeros((128, npairs * 64), ml_dtypes.float8_e4m3)
        li = pc["lane_of"]
        gg = li // GL
        lane = li % GL
        part = (gg % 2) * 64 + pc["rank_of"]
        colix = (gg // 2) * 64 + lane
        g3[part, colix] = 1.0

        # lane-indexed dest + val (pad lanes -> dummy rows, val 0)
        dest = np.full(nlanes, 0, np.int64)
        dest[:] = dummy0 + (np.arange(nlanes) % 32)
        val = np.zeros(nlanes, np.float32)
        dest[li] = pc["dests"]
        val[li] = pc["vals"]

        # idx wrap [128, nlanes/16] int16 (16-partition wrap, replicated x8)
        idxw = np.zeros((128, nlanes // 16), np.int16)
        w = dest.astype(np.int16).reshape(nlanes // 16, 16).T
        idxw[:] = np.tile(w, (8, 1))

        # val [128, NCALLS*16] bf16: [p, call*16+s] = val[call*CAP+128*s+p]
        vv = val.reshape(NCALLS, 16, 128).transpose(2, 0, 1).reshape(128, NCALLS * 16)
        valw = vv.astype(ml_dtypes.bfloat16)

        out.append({"xin": xin, "g3": g3, "idx": idxw, "val": valw})
    return out


def _make_bacc(ncores):
    from concourse import bacc
    return bacc.Bacc(
        "TRN2", target_bir_lowering=False, debug=False, num_devices=ncores,
        dynamic_dma_scratch_size=98304,
    )


def _build_program(nc, sched, n_nodes, fastpath, debug_agg=False):
    from contextlib import ExitStack
    import concourse.bass as bass
    import concourse.tile as tile
    from concourse import mybir

    f32 = mybir.dt.float32
    bf16 = mybir.dt.bfloat16
    fp8 = mybir.dt.float8e4
    i16 = mybir.dt.int16
    AF = mybir.ActivationFunctionType

    rpc = sched["rpc"]
    G = sched["G"]
    nlanes = sched["nlanes"]
    npairs = G // 2
    nrows_pad = sched["nrows_pad"]

    xind = nc.dram_tensor("xin", [128, npairs * 64], bf16, kind="ExternalInput")
    g3d = nc.dram_tensor("g3", [128, npairs * 64], fp8, kind="ExternalInput")
    idxd = nc.dram_tensor("idx", [128, nlanes // 16], i16, kind="ExternalInput")
    vald = nc.dram_tensor("val", [128, NCALLS * 16], bf16, kind="ExternalInput")
    wtbd = nc.dram_tensor("wtb", [DIM + 1, DIM], f32, kind="ExternalInput")
    gbd = nc.dram_tensor("gb", [2, DIM], f32, kind="ExternalInput")
    identd = nc.dram_tensor("ident", [128, 128], bf16, kind="ExternalInput")
    outd = nc.dram_tensor("out", [nrows_pad, DIM], f32, kind="ExternalOutput")

    nchunks = NCALLS // CHC
    PAIR_C = CHC * GPCALL // 2          # pairs per chunk (64)

    with tile.TileContext(nc) as tc, ExitStack() as ctx:
        singles = ctx.enter_context(tc.tile_pool(name="singles", bufs=1))
        xpool = ctx.enter_context(tc.tile_pool(name="xp", bufs=3))
        gpool = ctx.enter_context(tc.tile_pool(name="gp", bufs=3))
        ipool = ctx.enter_context(tc.tile_pool(name="ip", bufs=3))
        vpool = ctx.enter_context(tc.tile_pool(name="vp", bufs=3))
        mpool = ctx.enter_context(tc.tile_pool(name="mp", bufs=6))
        pagg = ctx.enter_context(tc.tile_pool(name="pagg", bufs=2, space="PSUM"))
        ptr = ctx.enter_context(tc.tile_pool(name="ptr", bufs=2, space="PSUM"))
        ph = ctx.enter_context(tc.tile_pool(name="ph", bufs=2, space="PSUM"))
        lpool = ctx.enter_context(tc.tile_pool(name="lp", bufs=2))

        wtb_s = singles.tile([DIM + 1, DIM], f32)
        nc.sync.dma_start(out=wtb_s[:], in_=wtbd[:])
        ident_s = singles.tile([128, 128], bf16)
        nc.sync.dma_start(out=ident_s[:], in_=identd[:])
        eps_s = singles.tile([128, 1], f32)
        nc.vector.memset(eps_s[:], LN_EPS)
        if not fastpath:
            gam_s = singles.tile([128, DIM], f32)
            bet_s = singles.tile([128, DIM], f32)
            gsrc = gbd.ap()
            nc.sync.dma_start(
                out=gam_s[:],
                in_=bass.AP(tensor=gsrc.tensor, offset=0, ap=[[0, 128], [1, DIM]]),
            )
            nc.sync.dma_start(
                out=bet_s[:],
                in_=bass.AP(tensor=gsrc.tensor, offset=DIM, ap=[[0, 128], [1, DIM]]),
            )

        # SBUF-resident agg tables (parity split), zeroed. KTAB independent
        # pairs used round-robin by scatter calls: breaks the WAW chain so
        # desc-gen/transfer of consecutive scatters overlap, and halves the
        # bf16 accumulation depth per table.
        KTAB = 4
        nblk = nrows_pad // 256
        agg_e = [
            singles.tile([128, nblk, DIM], bf16, name=f"agge{j}")
            for j in range(KTAB)
        ]
        agg_o = [
            singles.tile([128, nblk, DIM], bf16, name=f"aggo{j}")
            for j in range(KTAB)
        ]
        for t_ in agg_e + agg_o:
            nc.vector.memset(t_[:], 0.0)

        for ch in range(nchunks):
            c0 = ch * PAIR_C * 64
            c1 = (ch + 1) * PAIR_C * 64
            xt = xpool.tile([128, PAIR_C * 64], bf16, tag="xt")
            nc.sync.dma_start(out=xt[0:WIN, :], in_=xind[0:WIN, c0:c1])
            nc.sync.dma_start(out=xt[64:64 + WIN, :], in_=xind[64:64 + WIN, c0:c1])
            gt = gpool.tile([128, PAIR_C * 64], fp8, tag="gt")
            nc.sync.dma_start(out=gt[0:WIN, :], in_=g3d[0:WIN, c0:c1])
            nc.sync.dma_start(out=gt[64:64 + WIN, :], in_=g3d[64:64 + WIN, c0:c1])
            it = ipool.tile([128, CHC * CAP // 16], i16, tag="it")
            nc.sync.dma_start(
                out=it[:],
                in_=idxd[:, ch * CHC * CAP // 16:(ch + 1) * CHC * CAP // 16],
            )
            vt = vpool.tile([128, CHC * 16], bf16, tag="vt")
            nc.sync.dma_start(
                out=vt[:], in_=vald[:, ch * CHC * 16:(ch + 1) * CHC * 16]
            )

            for q in range(CHC):
                call = ch * CHC + q
                msgs = mpool.tile([128, 16, DIM], bf16, tag="msgs")
                ps = pagg.tile([128, 16, DIM], f32, tag="ps")
                for k in range(16):         # pair within call
                    pr = q * 16 + k         # pair within chunk
                    for h in range(2):      # group parity
                        po = 64 * h
                        nc.tensor.matmul(
                            out=ps[po:po + 64, k, :],
                            lhsT=gt[po:po + WIN, pr * 64:(pr + 1) * 64],
                            rhs=xt[po:po + WIN, pr * 64:(pr + 1) * 64],
                            start=True,
                            stop=True,
                            skip_group_check=True,
                            tile_position=(po, po),
                        )
                nc.scalar.copy(out=msgs[:], in_=ps[:])

                _vs = vt[:, q * 16:(q + 1) * 16]
                val_b = bass.AP(
                    tensor=_vs.tensor,
                    offset=_vs.offset,
                    ap=list(_vs.ap) + [[0, DIM]],
                )
                nc.vector.tensor_mul(out=msgs[:], in0=msgs[:], in1=val_b)

                j = call % KTAB
                nc.gpsimd.dma_scatter_add(
                    out_ap=agg_e[j][:],
                    out_ap_other=agg_o[j][:],
                    in_ap=msgs[:],
                    idxs_ap=it[:, q * (CAP // 16):(q + 1) * (CAP // 16)],
                    num_idxs=CAP,
                    num_idxs_reg=CAP,
                    elem_size=DIM,
                    sbuf_tokens_per_rank=128,
                    parity_reg=0,
                )

        if debug_agg:
            agge_d = nc.dram_tensor(
                "dbg_agge", [128, KTAB * nblk * DIM], bf16, kind="ExternalOutput"
            )
            aggo_d = nc.dram_tensor(
                "dbg_aggo", [128, KTAB * nblk * DIM], bf16, kind="ExternalOutput"
            )
            for j in range(KTAB):
                s0 = j * nblk * DIM
                s1 = (j + 1) * nblk * DIM
                nc.sync.dma_start(
                    out=agge_d[:, s0:s1],
                    in_=agg_e[j][:].rearrange("p b f -> p (b f)"),
                )
                nc.sync.dma_start(
                    out=aggo_d[:, s0:s1],
                    in_=agg_o[j][:].rearrange("p b f -> p (b f)"),
                )

        # combine the KTAB table pairs in place: (0+=1), (2+=3), (0+=2)
        for tabs in (agg_e, agg_o):
            nc.vector.tensor_add(out=tabs[0][:], in0=tabs[0][:], in1=tabs[1][:])
            nc.vector.tensor_add(out=tabs[2][:], in0=tabs[2][:], in1=tabs[3][:])
            nc.vector.tensor_add(out=tabs[0][:], in0=tabs[0][:], in1=tabs[2][:])

        # ---- epilogue: Linear + LayerNorm + ReLU per 128-row tile ----
        ntile128 = nrows_pad // 128
        ssq = singles.tile([128, ntile128], f32)
        rstd = singles.tile([128, ntile128], f32)
        sq_scratch = singles.tile([128, DIM], f32)

        BT = 7  # tiles per batch (98 tiles = 14 * 7)
        nbatch = ntile128 // BT
        assert nbatch * BT == ntile128, (ntile128, BT)
        for bt in range(nbatch):
            hps = ph.tile([128, BT, DIM], f32, tag="h")
            o_t = lpool.tile([128, BT, DIM], f32, tag="ot")
            for k in range(BT):
                t = bt * BT + k
                g, par = t // 2, t % 2
                src = agg_e[0] if par == 0 else agg_o[0]
                tp = ptr.tile([64, 128], bf16, tag="tp")
                nc.tensor.transpose(tp[:], src[:, g, :], ident_s[:])
                abT = lpool.tile([DIM + 1, 128], f32, tag="abT")
                nc.scalar.copy(out=abT[0:DIM, :], in_=tp[:])
                nc.vector.memset(abT[DIM:DIM + 1, :], 1.0)
                nc.tensor.matmul(
                    out=hps[:, k, :],
                    lhsT=abT[:],
                    rhs=wtb_s[:],
                    start=True,
                    stop=True,
                    skip_group_check=True,
                )
                nc.scalar.activation(
                    out=sq_scratch[:],
                    in_=hps[:, k, :],
                    func=AF.Square,
                    accum_out=ssq[:, t:t + 1],
                )
            nc.scalar.activation(
                out=rstd[:, bt * BT:(bt + 1) * BT],
                in_=ssq[:, bt * BT:(bt + 1) * BT],
                func=AF.Sqrt,
                bias=eps_s[:, :],
                scale=1.0 / DIM,
            )
            nc.vector.reciprocal(
                out=rstd[:, bt * BT:(bt + 1) * BT],
                in_=rstd[:, bt * BT:(bt + 1) * BT],
            )
            for k in range(BT):
                t = bt * BT + k
                if fastpath:
                    nc.scalar.activation(
                        out=o_t[:, k, :],
                        in_=hps[:, k, :],
                        func=AF.Relu,
                        scale=rstd[:, t:t + 1],
                    )
                else:
                    nc.scalar.mul(
                        out=o_t[:, k, :], in_=hps[:, k, :], mul=rstd[:, t:t + 1]
                    )
                    nc.vector.tensor_mul(
                        out=o_t[:, k, :], in0=o_t[:, k, :], in1=gam_s[:, :]
                    )
                    nc.vector.tensor_add(
                        out=o_t[:, k, :], in0=o_t[:, k, :], in1=bet_s[:, :]
                    )
                    nc.vector.tensor_scalar_max(
                        out=o_t[:, k, :], in0=o_t[:, k, :], scalar1=0.0
                    )
            dst = outd[bt * BT * 128:(bt + 1) * BT * 128, :].rearrange(
                "(k p) f -> p k f", p=128
            )
            nc.sync.dma_start(out=dst, in_=o_t[:])


def _execute(inputs, n_nodes=N_NODES, ncores=NCORES, trace=False, trace_cores=None):
    from concourse.bass_utils import run_bass_kernel_spmd

    x = np.asarray(inputs["x"], np.float32)
    W = np.asarray(inputs["W"], np.float32)
    b = np.asarray(inputs["b"], np.float32)
    gamma = np.asarray(inputs["gamma"], np.float32)
    beta = np.asarray(inputs["beta"], np.float32)

    sched = _host_prep(
        inputs["edge_row"], inputs["edge_col"], inputs["edge_val"], n_nodes, ncores
    )
    arrs = _host_arrays(sched, x)

    WT = W.T.astype(np.float32)
    WTc = WT - WT.mean(axis=1, keepdims=True)
    bc = (b - b.mean()).astype(np.float32)
    wtb = np.concatenate([WTc, bc[None, :]], axis=0).astype(np.float32)
    gb = np.stack([gamma, beta], axis=0).astype(np.float32)
    ident = np.eye(128, dtype=ml_dtypes.bfloat16)

    fastpath = bool(np.all(gamma == 1.0) and np.all(beta == 0.0))

    nc = _make_bacc(ncores)
    _build_program(nc, sched, n_nodes, fastpath)
    nc.compile()

    in_maps = [
        {
            "xin": np.ascontiguousarray(arrs[c]["xin"]),
            "g3": np.ascontiguousarray(arrs[c]["g3"]),
            "idx": np.ascontiguousarray(arrs[c]["idx"]),
            "val": np.ascontiguousarray(arrs[c]["val"]),
            "wtb": wtb,
            "gb": gb,
            "ident": ident,
        }
        for c in range(ncores)
    ]
    r = run_bass_kernel_spmd(
        nc,
        in_maps,
        list(range(ncores)),
        trace=trace,
        trace_cores=trace_cores,
    )
    rpc = n_nodes // ncores
    out = np.concatenate(
        [r.results[c]["out"][:rpc] for c in range(ncores)], axis=0
    )
    return out.astype(np.float32), r


def kernel(**inputs):
    out, _ = _execute(inputs)
    return out


# revision 24
# speedup vs baseline: 1.6384x; 1.0131x over previous
"""GCN layer (SpMM + Linear + LayerNorm + ReLU) on 8 Trainium2 NeuronCores.

Strategy (node sharding, streaming matmul-gather + CCE scatter-add):
  - Core c owns dest rows [c*rpc, (c+1)*rpc). Its edges are assigned to 100
    scatter calls of 2048 lanes such that dest rows are UNIQUE within each
    call (the CCE scatter-add loses colliding read-modify-writes inside one
    call; max dest degree ~36 << 100 calls). Runs of equal source col stay
    contiguous, so each 64-lane group still has <= 48 distinct source rows,
    which form the group's private x-window (host-assembled, streamed bf16).
  - Gather: per group, ONE TensorE matmul  Xg[64 lanes, 64f] = G3.T @ xwin
    with G3 [48, 64] a host-built fp8 one-hot (col-rank -> lane). No per-edge
    DMA descriptors on the gather side.
  - Messages: PSUM -> SBUF bf16 copy (scalar engine), then one DVE multiply
    by per-edge val (broadcast over the 64 feature columns).
  - Aggregation: gpsimd dma_scatter_add (SBUF parity-split CCE add) into
    SBUF-resident agg tables keyed by local dest row: partition=d&127,
    parity table=bit7(d), free col=d>>8. 128B descriptors (64 bf16);
    dynamic_dma_scratch_size=98304 so 2048-desc calls fit the SWDGE ring.
  - Epilogue per 128-row tile: PE transpose (bf16) -> [65,128] lhsT with ones
    row; Linear via wtb matmul (centering folded into weights); var from
    Square-activation accumulate; out = relu(h * rstd) on the gamma=1/beta=0
    fast path (general path uses vector ops).
"""

import numpy as np
import ml_dtypes

N_NODES = 100000
DIM = 64
LN_EPS = 1e-5
NCORES = 8

GL = 64          # lanes (edges) per group
WIN = 48         # x-window rows per group (max distinct cols per group)
CAP = 2048       # lanes per scatter call
GPCALL = CAP // GL           # 32 groups per call
CHC = 4          # calls per DMA chunk
NPH = 2          # dest-range phases (epilogue of phase p overlaps phase p+1)
NCALLS_P = 52    # scatter calls per phase (dest-unique within each call)
NCALLS = NPH * NCALLS_P
PH_ROW0 = (0, 6400)          # phase dest-row base (256-aligned)
PH_NBLK = (25, 24)           # 256-row blocks per phase (12544 rows total)


def _host_prep(edge_row, edge_col, edge_val, n_nodes, ncores):
    rpc = n_nodes // ncores
    G = NCALLS * GPCALL
    nlanes = NCALLS * CAP

    er = np.asarray(edge_row).astype(np.int64)
    ec = np.asarray(edge_col).astype(np.int64)
    ev = np.asarray(edge_val).astype(np.float32)
    core = er // rpc

    percore = []
    for c in range(ncores):
        m = core == c
        cols_a = ec[m]
        dests_a = (er[m] - c * rpc).astype(np.int64)
        vals_a = ev[m].astype(np.float64)
        # merge duplicate (col, dest) pairs (sum vals)
        order = np.lexsort((dests_a, cols_a))
        cols_a, dests_a, vals_a = cols_a[order], dests_a[order], vals_a[order]
        key_new = np.concatenate(
            [[True], (np.diff(cols_a) != 0) | (np.diff(dests_a) != 0)]
        )
        gid = np.cumsum(key_new) - 1
        vsum = np.zeros(gid[-1] + 1, np.float64)
        np.add.at(vsum, gid, vals_a)
        cols_a = cols_a[key_new]
        dests_a = dests_a[key_new]
        vals_a = vsum.astype(np.float32)

        all_cols = []
        all_dests = []     # phase-rebased dest tokens
        all_vals = []
        all_lane = []
        all_rank = []
        wcol = np.zeros(G * WIN, np.int64)
        pad_dest = np.zeros(nlanes, np.int64)   # filled per call below

        for p in range(NPH):
            row0 = PH_ROW0[p]
            row1 = row0 + PH_NBLK[p] * 256
            sel = (dests_a >= row0) & (dests_a < row1)
            cols = cols_a[sel]
            dests = dests_a[sel] - row0
            vals = vals_a[sel]
            E = len(cols)
            assert E <= NCALLS_P * CAP, (E, NCALLS_P * CAP)
            call0 = p * NCALLS_P

            starts = np.nonzero(np.concatenate([[True], np.diff(cols) != 0]))[0]
            ends = np.concatenate([starts[1:], [E]])

            lane_of = np.empty(E, np.int64)
            rank_of = np.empty(E, np.int64)
            call_lanes = [0] * NCALLS_P
            call_dest = [set() for _ in range(NCALLS_P)]
            grp_l = [0] * NCALLS_P
            grp_d = [0] * NCALLS_P
            ptr = 0

            def place_edges(ci, idxs):
                l, d = grp_l[ci], grp_d[ci]
                pos = call_lanes[ci]
                col = cols[idxs[0]]
                i = 0
                while i < len(idxs):
                    if l == 0:
                        d = 0
                    if d + 1 > WIN:
                        pos += GL - l
                        l, d = 0, 0
                    g = (call0 + ci) * GPCALL + pos // GL
                    d += 1
                    wcol[g * WIN + d - 1] = col
                    take = min(len(idxs) - i, GL - l)
                    for k in range(take):
                        e = idxs[i + k]
                        lane_of[e] = (call0 + ci) * CAP + pos
                        rank_of[e] = d - 1
                        pos += 1
                    l += take
                    i += take
                    if l == GL:
                        l, d = 0, 0
                call_lanes[ci] = pos
                grp_l[ci], grp_d[ci] = l, d

            def fits(ci, n, dset):
                if call_lanes[ci] + n > CAP:
                    return False
                if call_dest[ci] & dset:
                    return False
                l, d = grp_l[ci], grp_d[ci]
                rem = n
                extra = 0
                while rem > 0:
                    if l == 0:
                        d = 0
                    if d + 1 > WIN:
                        extra += GL - l
                        l, d = 0, 0
                    d += 1
                    take = min(rem, GL - l)
                    l += take
                    rem -= take
                    if l == GL:
                        l, d = 0, 0
                return call_lanes[ci] + n + extra <= CAP

            for s, e in zip(starts, ends):
                idxs = np.arange(s, e)
                dset = set(dests[s:e].tolist())
                placed = False
                for t in range(NCALLS_P):
                    ci = (ptr + t) % NCALLS_P
                    if fits(ci, e - s, dset):
                        call_dest[ci] |= dset
                        place_edges(ci, idxs)
                        placed = True
                        ptr = (ptr + 1) % NCALLS_P
                        break
                if not placed:
                    for e1 in idxs:
                        d1 = int(dests[e1])
                        pl = False
                        for t in range(NCALLS_P):
                            ci = (ptr + t) % NCALLS_P
                            if fits(ci, 1, {d1}):
                                call_dest[ci].add(d1)
                                place_edges(ci, np.asarray([e1]))
                                pl = True
                                ptr = (ptr + 1) % NCALLS_P
                                break
                        assert pl, "scatter call overflow"

            # pad lanes: unused-in-call dest tokens (zero messages; must be
            # unique within the call so the CCE RMW never races a real add)
            ndest_p = PH_NBLK[p] * 256
            for ci in range(NCALLS_P):
                base = (call0 + ci) * CAP
                free = 0
                used = call_dest[ci]
                for pos in range(CAP):
                    pad_dest[base + pos] = 0  # provisional
                # only positions >= call_lanes[ci] and group-pad holes are
                # unassigned; fill ALL positions provisionally, real lanes
                # overwrite later in _host_arrays.
                cnt = 0
                for pos in range(CAP):
                    while free in used and free < ndest_p - 1:
                        free += 1
                    pad_dest[base + pos] = free
                    cnt += 1
                    free += 1
                    if free >= ndest_p:
                        free = 0
                        used = ()  # wrapped: beyond call size anyway

            all_cols.append(cols)
            all_dests.append(dests)
            all_vals.append(vals)
            all_lane.append(lane_of)
            all_rank.append(rank_of)

        percore.append({
            "cols": np.concatenate(all_cols),
            "dests": np.concatenate(all_dests),
            "vals": np.concatenate(all_vals),
            "lane_of": np.concatenate(all_lane),
            "rank_of": np.concatenate(all_rank),
            "wcol": wcol,
            "pad_dest": pad_dest,
        })

    ntiles = (rpc + 255) // 256
    return {
        "rpc": rpc, "G": G, "nlanes": nlanes,
        "ntiles": ntiles, "nrows_pad": ntiles * 256,
        "percore": percore,
    }


def _host_arrays(sched, x):
    """Per-core input arrays for the SPMD program."""
    rpc = sched["rpc"]
    G = sched["G"]
    nlanes = sched["nlanes"]
    npairs = G // 2

    xbf = x.astype(ml_dtypes.bfloat16)
    out = []
    for pc in sched["percore"]:
        # xin [128, npairs*64] bf16: pair k cols [64k,64k+64); parts[0:WIN]
        # group 2k window rows (x rows), parts[64:64+WIN] group 2k+1
        xin = np.zeros((128, npairs * 64), ml_dtypes.bfloat16)
        wrows = xbf[pc["wcol"]].reshape(G, WIN, 64)
        v = xin[0:WIN].reshape(WIN, npairs, 64)
        v[:] = wrows[0::2].transpose(1, 0, 2)
        v2 = xin[64:64 + WIN].reshape(WIN, npairs, 64)
        v2[:] = wrows[1::2].transpose(1, 0, 2)

        # g3 [128, npairs*64] fp8 one-hot
        g3 = np.zeros((128, npairs * 64), ml_dtypes.float8_e4m3)
        li = pc["lane_of"]
        gg = li // GL
        lane = li % GL
        part = (gg % 2) * 64 + pc["rank_of"]
        colix = (gg // 2) * 64 + lane
        g3[part, colix] = 1.0

        # lane-indexed dest + val (pad lanes -> unused-in-call tokens, val 0)
        dest = pc["pad_dest"].copy()
        val = np.zeros(nlanes, np.float32)
        dest[li] = pc["dests"]
        val[li] = pc["vals"]

        # idx wrap [128, nlanes/16] int16 (16-partition wrap, replicated x8)
        idxw = np.zeros((128, nlanes // 16), np.int16)
        w = dest.astype(np.int16).reshape(nlanes // 16, 16).T
        idxw[:] = np.tile(w, (8, 1))

        # val [128, NCALLS*16] bf16: [p, call*16+s] = val[call*CAP+128*s+p]
        vv = val.reshape(NCALLS, 16, 128).transpose(2, 0, 1).reshape(128, NCALLS * 16)
        valw = vv.astype(ml_dtypes.bfloat16)

        out.append({"xin": xin, "g3": g3, "idx": idxw, "val": valw})
    return out


def _make_bacc(ncores):
    from concourse import bacc
    return bacc.Bacc(
        "TRN2", target_bir_lowering=False, debug=False, num_devices=ncores,
        dynamic_dma_scratch_size=98304,
    )


def _build_program(nc, sched, n_nodes, fastpath, debug_agg=False):
    from contextlib import ExitStack
    import concourse.bass as bass
    import concourse.tile as tile
    from concourse import mybir

    f32 = mybir.dt.float32
    bf16 = mybir.dt.bfloat16
    fp8 = mybir.dt.float8e4
    i16 = mybir.dt.int16
    AF = mybir.ActivationFunctionType

    rpc = sched["rpc"]
    G = sched["G"]
    nlanes = sched["nlanes"]
    npairs = G // 2
    nrows_pad = sched["nrows_pad"]

    xind = nc.dram_tensor("xin", [128, npairs * 64], bf16, kind="ExternalInput")
    g3d = nc.dram_tensor("g3", [128, npairs * 64], fp8, kind="ExternalInput")
    idxd = nc.dram_tensor("idx", [128, nlanes // 16], i16, kind="ExternalInput")
    vald = nc.dram_tensor("val", [128, NCALLS * 16], bf16, kind="ExternalInput")
    wtbd = nc.dram_tensor("wtb", [DIM + 1, DIM], f32, kind="ExternalInput")
    gbd = nc.dram_tensor("gb", [2, DIM], f32, kind="ExternalInput")
    identd = nc.dram_tensor("ident", [128, 128], bf16, kind="ExternalInput")
    outd = nc.dram_tensor("out", [nrows_pad, DIM], f32, kind="ExternalOutput")

    nchunks = NCALLS // CHC
    PAIR_C = CHC * GPCALL // 2          # pairs per chunk (64)

    with tile.TileContext(nc) as tc, ExitStack() as ctx:
        singles = ctx.enter_context(tc.tile_pool(name="singles", bufs=1))
        xpool = ctx.enter_context(tc.tile_pool(name="xp", bufs=3))
        gpool = ctx.enter_context(tc.tile_pool(name="gp", bufs=3))
        ipool = ctx.enter_context(tc.tile_pool(name="ip", bufs=3))
        vpool = ctx.enter_context(tc.tile_pool(name="vp", bufs=3))
        mpool = ctx.enter_context(tc.tile_pool(name="mp", bufs=6))
        pagg = ctx.enter_context(tc.tile_pool(name="pagg", bufs=2, space="PSUM"))
        ptr = ctx.enter_context(tc.tile_pool(name="ptr", bufs=2, space="PSUM"))
        ph = ctx.enter_context(tc.tile_pool(name="ph", bufs=2, space="PSUM"))
        lpool = ctx.enter_context(tc.tile_pool(name="lp", bufs=2))

        wtb_s = singles.tile([DIM + 1, DIM], f32)
        nc.sync.dma_start(out=wtb_s[:], in_=wtbd[:])
        ident_s = singles.tile([128, 128], bf16)
        nc.sync.dma_start(out=ident_s[:], in_=identd[:])
        eps_s = singles.tile([128, 1], f32)
        nc.vector.memset(eps_s[:], LN_EPS)
        if not fastpath:
            gam_s = singles.tile([128, DIM], f32)
            bet_s = singles.tile([128, DIM], f32)
            gsrc = gbd.ap()
            nc.sync.dma_start(
                out=gam_s[:],
                in_=bass.AP(tensor=gsrc.tensor, offset=0, ap=[[0, 128], [1, DIM]]),
            )
            nc.sync.dma_start(
                out=bet_s[:],
                in_=bass.AP(tensor=gsrc.tensor, offset=DIM, ap=[[0, 128], [1, DIM]]),
            )

        # SBUF-resident agg tables (parity split), zeroed. Per phase, KTAB
        # independent pairs used round-robin by scatter calls: breaks the WAW
        # chain so desc-gen/transfer of consecutive scatters overlap, and
        # shortens the bf16 accumulation depth per table.
        KTAB = 4
        nblk = nrows_pad // 256
        agg_e = [
            [
                singles.tile([128, PH_NBLK[p], DIM], bf16, name=f"agge{p}_{j}")
                for j in range(KTAB)
            ]
            for p in range(NPH)
        ]
        agg_o = [
            [
                singles.tile([128, PH_NBLK[p], DIM], bf16, name=f"aggo{p}_{j}")
                for j in range(KTAB)
            ]
            for p in range(NPH)
        ]
        for pt_ in agg_e + agg_o:
            for t_ in pt_:
                nc.vector.memset(t_[:], 0.0)

        def emit_chunks(p):
            ch0 = p * NCALLS_P // CHC
            ch1 = (p + 1) * NCALLS_P // CHC
            for ch in range(ch0, ch1):
                c0 = ch * PAIR_C * 64
                c1 = (ch + 1) * PAIR_C * 64
                xt = xpool.tile([128, PAIR_C * 64], bf16, tag="xt", name="xt")
                nc.sync.dma_start(out=xt[0:WIN, :], in_=xind[0:WIN, c0:c1])
                nc.sync.dma_start(
                    out=xt[64:64 + WIN, :], in_=xind[64:64 + WIN, c0:c1]
                )
                gt = gpool.tile([128, PAIR_C * 64], fp8, tag="gt", name="gt")
                nc.sync.dma_start(out=gt[0:WIN, :], in_=g3d[0:WIN, c0:c1])
                nc.sync.dma_start(out=gt[64:64 + WIN, :], in_=g3d[64:64 + WIN, c0:c1])
                it = ipool.tile([128, CHC * CAP // 16], i16, tag="it", name="it")
                nc.sync.dma_start(
                    out=it[:],
                    in_=idxd[:, ch * CHC * CAP // 16:(ch + 1) * CHC * CAP // 16],
                )
                vt = vpool.tile([128, CHC * 16], bf16, tag="vt", name="vt")
                nc.sync.dma_start(
                    out=vt[:], in_=vald[:, ch * CHC * 16:(ch + 1) * CHC * 16]
                )

                for q in range(CHC):
                    call = ch * CHC + q
                    msgs = mpool.tile([128, 16, DIM], bf16, tag="msgs", name="msgs")
                    ps = pagg.tile([128, 16, DIM], f32, tag="ps", name="ps")
                    for k in range(16):         # pair within call
                        pr = q * 16 + k         # pair within chunk
                        for h in range(2):      # group parity
                            po = 64 * h
                            nc.tensor.matmul(
                                out=ps[po:po + 64, k, :],
                                lhsT=gt[po:po + WIN, pr * 64:(pr + 1) * 64],
                                rhs=xt[po:po + WIN, pr * 64:(pr + 1) * 64],
                                start=True,
                                stop=True,
                                skip_group_check=True,
                                tile_position=(po, po),
                            )
                    nc.scalar.copy(out=msgs[:], in_=ps[:])

                    _vs = vt[:, q * 16:(q + 1) * 16]
                    val_b = bass.AP(
                        tensor=_vs.tensor,
                        offset=_vs.offset,
                        ap=list(_vs.ap) + [[0, DIM]],
                    )
                    nc.vector.tensor_mul(out=msgs[:], in0=msgs[:], in1=val_b)

                    j = call % KTAB
                    nc.gpsimd.dma_scatter_add(
                        out_ap=agg_e[p][j][:],
                        out_ap_other=agg_o[p][j][:],
                        in_ap=msgs[:],
                        idxs_ap=it[:, q * (CAP // 16):(q + 1) * (CAP // 16)],
                        num_idxs=CAP,
                        num_idxs_reg=CAP,
                        elem_size=DIM,
                        sbuf_tokens_per_rank=128,
                        parity_reg=0,
                    )

        # ---- per-phase combine + epilogue ----
        ntile128 = nrows_pad // 128
        ssq = singles.tile([128, ntile128], f32)
        rstd = singles.tile([128, ntile128], f32)
        sq_scratch = singles.tile([128, DIM], f32)
        BT_P = (5, 6)   # 50 tiles = 10*5, 48 tiles = 8*6

        def emit_epilogue(p):
            # combine the KTAB table pairs in place: (0+=1), (2+=3), (0+=2)
            for tabs in (agg_e[p], agg_o[p]):
                nc.vector.tensor_add(
                    out=tabs[0][:], in0=tabs[0][:], in1=tabs[1][:]
                )
                nc.vector.tensor_add(
                    out=tabs[2][:], in0=tabs[2][:], in1=tabs[3][:]
                )
                nc.vector.tensor_add(
                    out=tabs[0][:], in0=tabs[0][:], in1=tabs[2][:]
                )

            BT = BT_P[p]
            ntile_p = PH_NBLK[p] * 2
            t0g = PH_ROW0[p] // 128
            nbatch = ntile_p // BT
            assert nbatch * BT == ntile_p, (ntile_p, BT)
            for bt in range(nbatch):
                hps = ph.tile([128, BT, DIM], f32, tag="h", name="hps")
                o_t = lpool.tile([128, BT, DIM], f32, tag="ot", name="o_t")
                for k in range(BT):
                    t = bt * BT + k
                    g, par = t // 2, t % 2
                    src = agg_e[p][0] if par == 0 else agg_o[p][0]
                    tp = ptr.tile([64, 128], bf16, tag="tp", name="tp")
                    nc.tensor.transpose(tp[:], src[:, g, :], ident_s[:])
                    abT = lpool.tile([DIM + 1, 128], f32, tag="abT", name="abT")
                    nc.scalar.copy(out=abT[0:DIM, :], in_=tp[:])
                    nc.vector.memset(abT[DIM:DIM + 1, :], 1.0)
                    nc.tensor.matmul(
                        out=hps[:, k, :],
                        lhsT=abT[:],
                        rhs=wtb_s[:],
                        start=True,
                        stop=True,
                        skip_group_check=True,
                    )
                    nc.scalar.activation(
                        out=sq_scratch[:],
                        in_=hps[:, k, :],
                        func=AF.Square,
                        accum_out=ssq[:, t0g + t:t0g + t + 1],
                    )
                nc.scalar.activation(
                    out=rstd[:, t0g + bt * BT:t0g + (bt + 1) * BT],
                    in_=ssq[:, t0g + bt * BT:t0g + (bt + 1) * BT],
                    func=AF.Sqrt,
                    bias=eps_s[:, :],
                    scale=1.0 / DIM,
                )
                nc.vector.reciprocal(
                    out=rstd[:, t0g + bt * BT:t0g + (bt + 1) * BT],
                    in_=rstd[:, t0g + bt * BT:t0g + (bt + 1) * BT],
                )
                for k in range(BT):
                    t = bt * BT + k
                    tg = t0g + t
                    if fastpath:
                        nc.scalar.activation(
                            out=o_t[:, k, :],
                            in_=hps[:, k, :],
                            func=AF.Relu,
                            scale=rstd[:, tg:tg + 1],
                        )
                    else:
                        nc.scalar.mul(
                            out=o_t[:, k, :],
                            in_=hps[:, k, :],
                            mul=rstd[:, tg:tg + 1],
                        )
                        nc.vector.tensor_mul(
                            out=o_t[:, k, :], in0=o_t[:, k, :], in1=gam_s[:, :]
                        )
                        nc.vector.tensor_add(
                            out=o_t[:, k, :], in0=o_t[:, k, :], in1=bet_s[:, :]
                        )
                        nc.vector.tensor_scalar_max(
                            out=o_t[:, k, :], in0=o_t[:, k, :], scalar1=0.0
                        )
                r0 = PH_ROW0[p] + bt * BT * 128
                dst = outd[r0:r0 + BT * 128, :].rearrange(
                    "(k p) f -> p k f", p=128
                )
                nc.sync.dma_start(out=dst, in_=o_t[:])

        for p in range(NPH):
            emit_chunks(p)
            emit_epilogue(p)


def _execute(inputs, n_nodes=N_NODES, ncores=NCORES, trace=False, trace_cores=None):
    from concourse.bass_utils import run_bass_kernel_spmd

    x = np.asarray(inputs["x"], np.float32)
    W = np.asarray(inputs["W"], np.float32)
    b = np.asarray(inputs["b"], np.float32)
    gamma = np.asarray(inputs["gamma"], np.float32)
    beta = np.asarray(inputs["beta"], np.float32)

    sched = _host_prep(
        inputs["edge_row"], inputs["edge_col"], inputs["edge_val"], n_nodes, ncores
    )
    arrs = _host_arrays(sched, x)

    WT = W.T.astype(np.float32)
    WTc = WT - WT.mean(axis=1, keepdims=True)
    bc = (b - b.mean()).astype(np.float32)
    wtb = np.concatenate([WTc, bc[None, :]], axis=0).astype(np.float32)
    gb = np.stack([gamma, beta], axis=0).astype(np.float32)
    ident = np.eye(128, dtype=ml_dtypes.bfloat16)

    fastpath = bool(np.all(gamma == 1.0) and np.all(beta == 0.0))

    nc = _make_bacc(ncores)
    _build_program(nc, sched, n_nodes, fastpath)
    nc.compile()

    in_maps = [
        {
            "xin": np.ascontiguousarray(arrs[c]["xin"]),
            "g3": np.ascontiguousarray(arrs[c]["g3"]),
            "idx": np.ascontiguousarray(arrs[c]["idx"]),
            "val": np.ascontiguousarray(arrs[c]["val"]),
            "wtb": wtb,
            "gb": gb,
            "ident": ident,
        }
        for c in range(ncores)
    ]
    r = run_bass_kernel_spmd(
        nc,
        in_maps,
        list(range(ncores)),
        trace=trace,
        trace_cores=trace_cores,
    )
    rpc = n_nodes // ncores
    out = np.concatenate(
        [r.results[c]["out"][:rpc] for c in range(ncores)], axis=0
    )
    return out.astype(np.float32), r


def kernel(**inputs):
    out, _ = _execute(inputs)
    return out
